# revision 53
# baseline (speedup 1.0000x reference)
"""Trainium2 Bass kernel for nn_GRIC_31550829756424 (GCN-attention block).

Data-parallel over batch: 8 batches -> 8 NeuronCores, one full batch per core.
417581 ns (baseline) -> 133927 ns (TimelineSim, per core).

Key structure (v3):
- GCN reassociated: adjHnT = Hn^T @ adj_norm^T computed once (shared by
  Q/K/V), then Q/K/V are single-step K=128 matmuls.  A is host-transposed,
  bf16; deg^-1/2 folds into hn_t (partition side) + the adjHnT PSUM move
  (free side), so A^T itself is never rescaled.
- One activation table for the whole kernel (natural_log_exp_and_others,
  preloaded explicitly): every rsqrt is exp(-0.5*ln(x)); only Exp / Ln /
  Relu / Copy / Identity are ever used, so zero mid-kernel table reloads.
- Attention bias B added into the QK PSUM by an fp8e4 DoubleRow matmul
  (I/32 stationary x2 slots, bias*16 moving with a stride-0 slot dim) at
  0.5 cyc/row; B_bias ships as fp8 (halves its DMA traffic).
- Attention software-pipelined over heads: stage h runs scores+Exp(h) on
  PE/ACT while PV+LN(h-1) runs on PE/DVE/Pool; LN mean-subtract frees PSUM
  immediately, rstd is batched per half-head, MH transposed by DMA XBAR.
- LN1 gain/bias folded into W_O (host); LN2 gain/bias folded into W1/b1
  (host); residual H and the W_O/V biases accumulate on the PE via
  identity/ones-row matmuls.

Self-contained: hardcodes all shapes; imports only the in-container
concourse stack.
"""

import sys

sys.path.insert(0, "/opt/trn_rl_repo")

import numpy as np
import ml_dtypes
from contextlib import ExitStack

import concourse.bass as bass
import concourse.tile as tile
from concourse import bacc
from concourse import mybir
from concourse.bass_utils import run_bass_kernel_spmd
from concourse.masks import make_identity

F32 = mybir.dt.float32
BF16 = mybir.dt.bfloat16
F8 = mybir.dt.float8e4
AF = mybir.ActivationFunctionType
OP = mybir.AluOpType
PM = mybir.MatmulPerfMode

B = 8
N = 1024
D = 128
HEADS = 8
DV = 128
HD = HEADS * DV  # 1024
P = 128
NT = N // P  # 8 tiles of 128 rows
DK = 1.0 / float(np.sqrt(np.float32(D)))
EPS = 1e-5

_prog_cache = {}


def _bcast_load(nc, dst, src):
    """DMA-load 1D DRAM vector src [W] replicated across all P partitions of
    dst [P, W] (same dtype)."""
    rep = bass.AP(tensor=src.tensor, offset=src.offset, ap=[[0, P]] + list(src.ap))
    nc.gpsimd.dma_start(out=dst, in_=rep)


def _dup2(ap):
    """View a [P, W] AP as [P, 2, W] with a stride-0 middle dim (DoubleRow
    moving operand reading the same data in both slots)."""
    return bass.AP(
        tensor=ap.tensor, offset=ap.offset,
        ap=[list(ap.ap[0]), [0, 2]] + [list(a) for a in ap.ap[1:]],
    )


def _build_program():
    nc = bacc.Bacc(None)

    h_in = nc.declare_dram_parameter("h", [N, D], BF16, isOutput=False)
    at_in = nc.declare_dram_parameter("at", [N, N], BF16, isOutput=False)
    bt_in = nc.declare_dram_parameter("bt", [HEADS, N, N], F8, isOutput=False)
    wq_in = nc.declare_dram_parameter("wq", [D, HD], BF16, isOutput=False)
    wk_in = nc.declare_dram_parameter("wk", [D, HD], BF16, isOutput=False)
    wv_in = nc.declare_dram_parameter("wv", [D, HD], BF16, isOutput=False)
    bqr_in = nc.declare_dram_parameter("bqr", [P, NT], F32, isOutput=False)
    bkr_in = nc.declare_dram_parameter("bkr", [P, NT], F32, isOutput=False)
    bv_in = nc.declare_dram_parameter("bv", [HD], F32, isOutput=False)
    wo_in = nc.declare_dram_parameter("wo", [HD, D], BF16, isOutput=False)
    bo_in = nc.declare_dram_parameter("bo", [1, D], BF16, isOutput=False)
    w1_in = nc.declare_dram_parameter("w1", [D, D], BF16, isOutput=False)
    w2_in = nc.declare_dram_parameter("w2", [D, D], BF16, isOutput=False)
    b1_in = nc.declare_dram_parameter("b1", [D, 1], F32, isOutput=False)
    b2_in = nc.declare_dram_parameter("b2", [D, 1], F32, isOutput=False)
    g_in = {}
    be_in = {}
    for i in (0, 2, 3):
        g_in[i] = nc.declare_dram_parameter(f"g{i}", [D], BF16, isOutput=False)
        be_in[i] = nc.declare_dram_parameter(f"be{i}", [D], BF16, isOutput=False)
    out_dram = nc.declare_dram_parameter("out", [N, D], F32, isOutput=True)

    with tile.TileContext(nc) as tc, ExitStack() as ctx:
        consts = ctx.enter_context(tc.tile_pool(name="consts", bufs=1))
        persist = ctx.enter_context(tc.tile_pool(name="persist", bufs=1))
        small = ctx.enter_context(tc.tile_pool(name="small", bufs=12))
        stg = ctx.enter_context(tc.tile_pool(name="stg", bufs=3))
        # 2-bank 512-wide PSUM pool: open through B/C, reused in E via scope.
        ps512 = ctx.enter_context(
            tc.tile_pool(name="ps512", bufs=2, space=bass.MemorySpace.PSUM))

        # ---- constants -------------------------------------------------
        identb = consts.tile([P, P], BF16)
        make_identity(nc, identb)
        omib = consts.tile([P, P], BF16)  # 1 - I
        nc.gpsimd.memset(omib, 1.0)
        nc.gpsimd.affine_select(
            out=omib, in_=omib, compare_op=OP.not_equal, fill=0.0,
            base=0, pattern=[[-1, P]], channel_multiplier=1)
        # fp8 DoubleRow stationary: two slots of I/32 (moving is bias*16).
        id2 = consts.tile([P, 2, P], F8)
        nc.gpsimd.memset(id2, 0.0)
        for s in range(2):
            nc.gpsimd.affine_select(
                out=id2[:, s, :], in_=id2[:, s, :], compare_op=OP.not_equal,
                fill=1.0 / 32.0, base=0, pattern=[[-1, P]], channel_multiplier=1)
        nc.scalar.add_instruction(
            mybir.InstLoadActFuncSet(
                name=nc.get_next_instruction_name(), ins=[], outs=[],
                act_func_set_id=6))
        eps_t = consts.tile([P, 1], F32)
        nc.vector.memset(eps_t, EPS)
        onesb = consts.tile([1, P], BF16)
        nc.vector.memset(onesb, 1.0)
        onescol = consts.tile([P, 1], BF16)
        nc.vector.memset(onescol, 1.0)

        gb = {}
        beb = {}
        for i in (0, 2, 3):
            gb[i] = consts.tile([P, D], BF16, name=f"g{i}b", tag=f"g{i}b")
            _bcast_load(nc, gb[i], g_in[i][:])
            beb[i] = consts.tile([P, D], BF16, name=f"be{i}b", tag=f"be{i}b")
            _bcast_load(nc, beb[i], be_in[i][:])
        bvb = consts.tile([P, HD], F32)
        _bcast_load(nc, bvb, bv_in[:])
        bqr = consts.tile([P, NT], F32)
        nc.gpsimd.dma_start(out=bqr, in_=bqr_in[:, :])
        bkr = consts.tile([P, NT], F32)
        nc.gpsimd.dma_start(out=bkr, in_=bkr_in[:, :])
        wq_sb = consts.tile([P, HD], BF16)
        nc.gpsimd.dma_start(out=wq_sb, in_=wq_in[:, :])
        wk_sb = consts.tile([P, HD], BF16)
        nc.gpsimd.dma_start(out=wk_sb, in_=wk_in[:, :])
        wv_sb = consts.tile([P, HD], BF16)
        nc.gpsimd.dma_start(out=wv_sb, in_=wv_in[:, :])
        # W_O as [p=dv-within-head, h, d]
        wo_sb = consts.tile([P, HEADS, D], BF16)
        nc.sync.dma_start(
            out=wo_sb, in_=wo_in.rearrange("(hh p) d -> p hh d", p=P))
        bo_sb = consts.tile([1, D], BF16)
        nc.gpsimd.dma_start(out=bo_sb, in_=bo_in[:, :])
        w1_sb = consts.tile([P, D], BF16)
        nc.gpsimd.dma_start(out=w1_sb, in_=w1_in[:, :])
        w2_sb = consts.tile([P, D], BF16)
        nc.gpsimd.dma_start(out=w2_sb, in_=w2_in[:, :])
        b1_sb = consts.tile([P, 1], F32)
        nc.gpsimd.dma_start(out=b1_sb, in_=b1_in[:, :])
        b2_sb = consts.tile([P, 1], F32)
        nc.gpsimd.dma_start(out=b2_sb, in_=b2_in[:, :])

        # ---- persistent tensors ---------------------------------------
        h_sb = persist.tile([P, NT, D], BF16, tag="h")
        hn_t = persist.tile([P, NT, D], BF16, tag="hnt")
        ats = persist.tile([P, NT, N], BF16, tag="ats")  # A^T -> adj_norm^T
        for j in range(NT):
            nc.sync.dma_start(
                out=ats[:, j, :],
                in_=at_in[:, :].rearrange("(t p) n -> p t n", p=P)[:, j, :])
        nc.sync.dma_start(out=h_sb, in_=h_in.rearrange("(t p) d -> p t d", p=P))
        disb = persist.tile([P, N], BF16, tag="disb")
        dis_tok = persist.tile([P, NT], F32, tag="distok")
        adjHnT = persist.tile([P, N], BF16, tag="adjhnt")
        qT = persist.tile([P, HEADS, N], BF16, tag="qT")
        kT = persist.tile([P, HEADS, N], BF16, tag="kT")
        vna = persist.tile([P, NT, HEADS, DV + 1], BF16, tag="v")
        nc.vector.memset(vna[:, :, :, DV:DV + 1], 1.0)
        mhcT = persist.tile([P, HEADS, N], BF16, tag="mhcT")

        # ---- phase B part 1: diagonal fix + rowsums ---------------------
        with tc.tile_pool(name="psR", bufs=1, space=bass.MemorySpace.PSUM) as psR, \
             tc.tile_pool(name="psTb", bufs=2, space=bass.MemorySpace.PSUM) as psTb:
            rs_ps = [psR.tile([1, 512], F32, name=f"rsps{c}", tag=f"rsps{c}")
                     for c in range(2)]
            for j in range(NT):
                db = ats[:, j, j * P:(j + 1) * P]
                nc.vector.tensor_mul(out=db, in0=db, in1=omib)
                nc.vector.tensor_add(out=db, in0=db, in1=identb)
                for c in range(2):
                    nc.tensor.matmul(
                        rs_ps[c], onescol, ats[:, j, c * 512:(c + 1) * 512],
                        start=(j == 0), stop=(j == NT - 1))
            # -- phase A: H LayerNorm (interleaved) --
            for i in range(NT):
                s6 = small.tile([P, 6], F32, tag="s6")
                mv = small.tile([P, 2], F32, tag="mv")
                nc.vector.bn_stats(out=s6, in_=h_sb[:, i, :])
                nc.vector.bn_aggr(out=mv, in_=s6)
                lnv = small.tile([P, 1], F32, tag="lnv")
                nc.scalar.activation(out=lnv, in_=mv[:, 1:2], func=AF.Ln, bias=eps_t)
                rstd = small.tile([P, 1], F32, tag="rstd")
                nc.scalar.activation(out=rstd, in_=lnv, func=AF.Exp, scale=-0.5)
                hw = stg.tile([P, D], BF16, name="hw", tag="hw")
                nc.vector.tensor_scalar(
                    out=hw, in0=h_sb[:, i, :], scalar1=mv[:, 0:1], scalar2=rstd,
                    op0=OP.subtract, op1=OP.mult)
                nc.vector.tensor_mul(out=hw, in0=hw, in1=gb[0])
                nc.vector.tensor_add(out=hn_t[:, i, :], in0=hw, in1=beb[0])

            rs_sb = small.tile([1, N], F32, tag="rssb", bufs=1)
            for c in range(2):
                nc.vector.tensor_scalar_max(
                    out=rs_sb[:, c * 512:(c + 1) * 512], in0=rs_ps[c], scalar1=1.0)
            lnr = small.tile([1, N], F32, tag="lnr", bufs=1)
            nc.scalar.activation(out=lnr, in_=rs_sb, func=AF.Ln)
            disrow = small.tile([1, N], BF16, tag="disrow", bufs=1)
            nc.scalar.activation(out=disrow, in_=lnr, func=AF.Exp, scale=-0.5)
            # broadcast di over partitions: disb[q, n] = di_n
            for c in range(2):
                psd = ps512.tile([P, 512], F32, tag="ps512")
                nc.tensor.matmul(
                    psd, onesb, disrow[:, c * 512:(c + 1) * 512],
                    start=True, stop=True)
                nc.scalar.activation(
                    out=disb[:, c * 512:(c + 1) * 512], in_=psd, func=AF.Copy)
            # dis_tok[p, j] = di_{j*P+p} via PE transpose of disb chunks
            for j in range(NT):
                ptb = psTb.tile([P, P], BF16, tag="ptb")
                nc.tensor.transpose(ptb, disb[:, j * P:(j + 1) * P], identb)
                nc.scalar.activation(out=dis_tok[:, j:j + 1], in_=ptb[:, 0:1],
                                     func=AF.Copy)
            # fold di_m into hn_t (per-partition) instead of scaling ats
            for j in range(NT):
                nc.vector.tensor_scalar_mul(
                    out=hn_t[:, j, :], in0=hn_t[:, j, :],
                    scalar1=dis_tok[:, j:j + 1])

        # ---- phase C: shared GCN trunk ----------------------------------
        # adjHnT[d, n] = sum_m hn[m, d] * adjn^T[m, n]
        for c in range(2):
            psc = ps512.tile([P, 512], F32, tag="ps512")
            for j in range(NT):
                nc.tensor.matmul(
                    psc, hn_t[:, j, :], ats[:, j, c * 512:(c + 1) * 512],
                    start=(j == 0), stop=(j == NT - 1))
            nc.vector.tensor_mul(
                out=adjHnT[:, c * 512:(c + 1) * 512], in0=psc,
                in1=disb[:, c * 512:(c + 1) * 512])

        def emit_qk(hh, q_on_act=False):
            for c in range(2):
                psc = ps512.tile([P, 512], F32, tag="ps512")
                nc.tensor.matmul(
                    psc, wq_sb[:, hh * P:(hh + 1) * P],
                    adjHnT[:, c * 512:(c + 1) * 512], start=True, stop=True)
                if q_on_act:
                    nc.scalar.activation(
                        out=qT[:, hh, c * 512:(c + 1) * 512], in_=psc,
                        func=AF.Identity, bias=bqr[:, hh:hh + 1])
                else:
                    nc.vector.tensor_scalar_add(
                        out=qT[:, hh, c * 512:(c + 1) * 512], in0=psc,
                        scalar1=bqr[:, hh:hh + 1])
                psc = ps512.tile([P, 512], F32, tag="ps512")
                nc.tensor.matmul(
                    psc, wk_sb[:, hh * P:(hh + 1) * P],
                    adjHnT[:, c * 512:(c + 1) * 512], start=True, stop=True)
                nc.vector.tensor_scalar_add(
                    out=kT[:, hh, c * 512:(c + 1) * 512], in0=psc,
                    scalar1=bkr[:, hh:hh + 1])

        emit_qk(0, q_on_act=True)

        def emit_v(c, i0=0, i1=NT):
            for i in range(i0, i1):
                psc = ps512.tile([P, 512], F32, tag="ps512")
                nc.tensor.matmul(
                    psc, adjHnT[:, i * P:(i + 1) * P],
                    wv_sb[:, c * 512:(c + 1) * 512], start=True, stop=True)
                nc.vector.tensor_add(
                    out=vna[:, i, c * 4:(c + 1) * 4, 0:DV],
                    in0=psc.rearrange("p (a b) -> p a b", a=4),
                    in1=bvb[:, c * 512:(c + 1) * 512].rearrange(
                        "p (a b) -> p a b", a=4))

        emit_v(0)

        # ---- phase D: attention, software-pipelined over heads ----------
        with tc.tile_pool(name="psE", bufs=2, space=bass.MemorySpace.PSUM) as psE, \
             tc.tile_pool(name="psPM", bufs=2, space=bass.MemorySpace.PSUM) as psPM, \
             tc.tile_pool(name="etp", bufs=3) as etp, \
             tc.tile_pool(name="btp", bufs=2) as btp, \
             tc.tile_pool(name="mhp", bufs=3) as mhp:
            ets = {}
            mhs = {}
            mvss = {}
            vees = {}
            bt0 = btp.tile([P, NT, N], F8, tag="bt", name="bt0")
            nc.sync.dma_start(
                out=bt0, in_=bt_in[0].rearrange("(t p) n -> p t n", p=P))
            bts = {0: bt0}
            for stage in range(HEADS + 1):
                hh = stage
                if hh < HEADS:
                    et = etp.tile([P, NT, N], BF16, tag="et")
                    ets[hh] = et
                    btile = bts[hh]
                    for j in range(NT):
                        pse = psE.tile([P, N], F32, tag="pse")
                        for c in range(2):
                            nc.tensor.matmul(
                                pse[:, c * 512:(c + 1) * 512],
                                kT[:, hh, j * P:(j + 1) * P],
                                qT[:, hh, c * 512:(c + 1) * 512],
                                start=True, stop=False)
                            nc.tensor.matmul(
                                pse[:, c * 512:(c + 1) * 512],
                                id2, _dup2(btile[:, j, c * 512:(c + 1) * 512]),
                                start=False, stop=True, perf_mode=PM.DoubleRow)
                        nc.scalar.activation(out=et[:, j, :], in_=pse,
                                             func=AF.Exp)
                    if hh + 1 < HEADS:
                        btn = btp.tile([P, NT, N], F8, tag="bt",
                                       name=f"bt{hh + 1}")
                        nc.sync.dma_start(
                            out=btn,
                            in_=bt_in[hh + 1].rearrange("(t p) n -> p t n", p=P))
                        bts[hh + 1] = btn
                        if hh + 1 < HEADS:
                            emit_qk(hh + 1)
                if 1 <= stage <= 4:
                    emit_v(1, (stage - 1) * 2, stage * 2)
                if stage >= 1:
                    ph = stage - 1  # head whose PV/LN we process now
                    et = ets.pop(ph)
                    mh_sub = mhp.tile([P, NT, DV], BF16, tag="mh")
                    mhs[ph] = mh_sub
                    mvs = small.tile([P, NT, 2], F32, tag="mvs", bufs=2)
                    mvss[ph] = mvs
                    vee = small.tile([P, NT], F32, tag="vee", bufs=2)
                    vees[ph] = vee
                    for i in range(NT):
                        pm = psPM.tile([P, DV + 1], F32, tag="pm")
                        for j in range(NT):
                            nc.tensor.matmul(
                                pm, et[:, j, i * P:(i + 1) * P],
                                vna[:, j, ph, :],
                                start=(j == 0), stop=(j == NT - 1))
                        s6 = small.tile([P, 6], F32, tag="s6")
                        nc.vector.bn_stats(out=s6, in_=pm[:, 0:DV])
                        nc.vector.bn_aggr(out=mvs[:, i, :], in_=s6)
                        t = small.tile([P, 1], F32, tag="t")
                        nc.vector.tensor_scalar(
                            out=t, in0=pm[:, DV:DV + 1],
                            scalar1=pm[:, DV:DV + 1], scalar2=EPS,
                            op0=OP.mult, op1=OP.mult)
                        nc.gpsimd.tensor_add(
                            out=vee[:, i:i + 1], in0=t, in1=mvs[:, i, 1:2])
                        # mean-subtract now (frees pm); rstd scale later
                        nc.vector.tensor_scalar(
                            out=mh_sub[:, i, :], in0=pm[:, 0:DV],
                            scalar1=mvs[:, i, 0:1], scalar2=None,
                            op0=OP.subtract)
                    # rstd batched per half-head: first scales/transposes
                    # start before the second half's PV stats finish
                    lnv8 = small.tile([P, NT], F32, tag="lnv8", bufs=2)
                    rstd8 = small.tile([P, NT], F32, tag="rstd8", bufs=2)
                    eng = nc.vector if ph == HEADS - 1 else nc.gpsimd
                    for half in range(2):
                        sl = slice(half * 4, (half + 1) * 4)
                        nc.scalar.activation(out=lnv8[:, sl], in_=vee[:, sl],
                                             func=AF.Ln)
                        nc.scalar.activation(out=rstd8[:, sl], in_=lnv8[:, sl],
                                             func=AF.Exp, scale=-0.5)
                        for i in range(half * 4, (half + 1) * 4):
                            eng.tensor_scalar_mul(
                                out=mh_sub[:, i, :], in0=mh_sub[:, i, :],
                                scalar1=rstd8[:, i:i + 1])
                        for qt in range(half * 2, (half + 1) * 2):
                            nc.sync.dma_start(
                                out=mhcT[:, ph, qt * 256:(qt + 1) * 256]
                                .rearrange("p (t f) -> p t f", t=2),
                                in_=mh_sub[:, qt * 2:(qt + 1) * 2, :],
                                transpose=True)

        # ---- phase E: output projection + MLP ---------------------------
        o_ln = persist.tile([P, NT, D], BF16, tag="oln")
        obe = persist.tile([P, NT, D], BF16, tag="obe")
        mvE = small.tile([P, NT, 2], F32, tag="mvE", bufs=1)
        rstdE = small.tile([P, NT], F32, tag="rstdE", bufs=1)
        with tc.tile_pool(name="psO", bufs=4, space=bass.MemorySpace.PSUM) as psO, \
             tc.tile_pool(name="psTe", bufs=2, space=bass.MemorySpace.PSUM) as psTe:
            oT = persist.tile([P, NT, P], BF16, tag="oT")
            for i in range(NT):
                pso = psO.tile([P, D], F32, tag="pso")
                for hh in range(HEADS):
                    nc.tensor.matmul(
                        pso, mhcT[:, hh, i * P:(i + 1) * P], wo_sb[:, hh, :],
                        start=(hh == 0), stop=False)
                nc.tensor.matmul(pso, onesb, bo_sb, start=False, stop=False)
                nc.tensor.matmul(pso, identb, h_sb[:, i, :],
                                 start=False, stop=True)
                s6 = small.tile([P, 6], F32, tag="s6")
                nc.vector.bn_stats(out=s6, in_=pso)
                nc.vector.bn_aggr(out=mvE[:, i, :], in_=s6)
                lnv = small.tile([P, 1], F32, tag="lnv")
                nc.scalar.activation(out=lnv, in_=mvE[:, i, 1:2], func=AF.Ln,
                                     bias=eps_t)
                nc.scalar.activation(out=rstdE[:, i:i + 1], in_=lnv,
                                     func=AF.Exp, scale=-0.5)
                nc.vector.tensor_scalar(
                    out=o_ln[:, i, :], in0=pso,
                    scalar1=mvE[:, i, 0:1], scalar2=rstdE[:, i:i + 1],
                    op0=OP.subtract, op1=OP.mult)
                ptb = psTe.tile([P, P], BF16, tag="pte")
                nc.tensor.transpose(ptb, o_ln[:, i, :], identb)
                nc.vector.tensor_copy(out=oT[:, i, :], in_=ptb)
                nc.gpsimd.tensor_mul(out=obe[:, i, :], in0=o_ln[:, i, :],
                                     in1=gb[2])
                nc.gpsimd.tensor_add(out=obe[:, i, :], in0=obe[:, i, :],
                                     in1=beb[2])
            r1T = persist.tile([P, N], BF16, tag="r1T")
            for c in range(2):
                psc = ps512.tile([P, 512], F32, tag="ps512")
                nc.tensor.matmul(
                    psc, w1_sb,
                    oT[:, 4 * c:4 * (c + 1), :].rearrange("p t f -> p (t f)"),
                    start=True, stop=True)
                nc.scalar.activation(
                    out=r1T[:, c * 512:(c + 1) * 512], in_=psc, func=AF.Relu,
                    bias=b1_sb)
            r2T = persist.tile([P, N], BF16, tag="r2T")
            for c in range(2):
                psc = ps512.tile([P, 512], F32, tag="ps512")
                nc.tensor.matmul(
                    psc, w2_sb, r1T[:, c * 512:(c + 1) * 512],
                    start=True, stop=True)
                nc.scalar.activation(
                    out=r2T[:, c * 512:(c + 1) * 512], in_=psc, func=AF.Relu,
                    bias=b2_sb)

            r2tok = persist.tile([P, NT, D], BF16, tag="r2tok")
            mvR = small.tile([P, NT, 2], F32, tag="mvR", bufs=1)
            rstdR = small.tile([P, NT], F32, tag="rstdR", bufs=1)
            out_sb = persist.tile([P, NT, D], F32, tag="osb")
            for i in range(NT):
                ptb = psTe.tile([P, P], BF16, tag="pte")
                nc.tensor.transpose(ptb, r2T[:, i * P:(i + 1) * P], identb)
                nc.scalar.activation(out=r2tok[:, i, :], in_=ptb, func=AF.Copy)
                s6 = small.tile([P, 6], F32, tag="s6")
                nc.vector.bn_stats(out=s6, in_=r2tok[:, i, :])
                nc.vector.bn_aggr(out=mvR[:, i, :], in_=s6)
                lnv = small.tile([P, 1], F32, tag="lnv")
                nc.scalar.activation(out=lnv, in_=mvR[:, i, 1:2], func=AF.Ln,
                                     bias=eps_t)
                nc.scalar.activation(out=rstdR[:, i:i + 1], in_=lnv,
                                     func=AF.Exp, scale=-0.5)
            for i in range(NT):
                ro = stg.tile([P, D], BF16, name="ro", tag="ro")
                nc.vector.tensor_scalar(
                    out=ro, in0=r2tok[:, i, :], scalar1=mvR[:, i, 0:1],
                    scalar2=rstdR[:, i:i + 1], op0=OP.subtract, op1=OP.mult)
                nc.vector.tensor_mul(out=ro, in0=ro, in1=gb[3])
                nc.vector.tensor_add(out=out_sb[:, i, :], in0=obe[:, i, :],
                                     in1=ro)
                if i % 4 == 3:
                    nc.sync.dma_start(
                        out=out_dram.rearrange(
                            "(t p) d -> p t d", p=P)[:, i - 3:i + 1, :],
                        in_=out_sb[:, i - 3:i + 1, :])

    nc.compile()
    return nc


def _get_program():
    if "nc" not in _prog_cache:
        _prog_cache["nc"] = _build_program()
    return _prog_cache["nc"]


def kernel(**inputs):
    nc = _get_program()
    f32 = np.float32
    bf16 = ml_dtypes.bfloat16
    f8 = ml_dtypes.float8_e4m3fn

    H = np.asarray(inputs["H"], dtype=f32)
    A = np.asarray(inputs["A"], dtype=f32)
    g1 = np.asarray(inputs["g1"], dtype=f32)
    be1 = np.asarray(inputs["be1"], dtype=f32)
    WO = np.asarray(inputs["W_O"], dtype=f32)
    # fold LN1 gain/bias into the output projection
    WO_fold = WO * np.tile(g1, HEADS)[:, None]
    bO = np.tile(be1, HEADS) @ WO

    BT = np.asarray(inputs["B_bias"], dtype=f32).transpose(0, 2, 1)
    base = {
        "bt": np.ascontiguousarray(BT * 16.0).astype(f8),
        "wq": (np.asarray(inputs["W_Q"], dtype=f32) * DK).astype(bf16),
        "wk": np.asarray(inputs["W_K"], dtype=f32).astype(bf16),
        "wv": np.asarray(inputs["W_V"], dtype=f32).astype(bf16),
        "bqr": np.ascontiguousarray(
            (np.asarray(inputs["b_Q"], dtype=f32) * DK).reshape(NT, P).T),
        "bkr": np.ascontiguousarray(
            np.asarray(inputs["b_K"], dtype=f32).reshape(NT, P).T),
        "bv": np.asarray(inputs["b_V"], dtype=f32),
        "wo": WO_fold.astype(bf16),
        "bo": bO.reshape(1, D).astype(bf16),
        "w1": (np.asarray(inputs["g2"], dtype=f32)[:, None]
               * np.asarray(inputs["W1"], dtype=f32)).astype(bf16),
        "w2": np.asarray(inputs["W2"], dtype=f32).astype(bf16),
        "b1": (np.asarray(inputs["b1"], dtype=f32)
               + np.asarray(inputs["be2"], dtype=f32)
               @ np.asarray(inputs["W1"], dtype=f32)).reshape(D, 1),
        "b2": np.asarray(inputs["b2"], dtype=f32).reshape(D, 1),
    }
    for i in (0, 2, 3):
        base[f"g{i}"] = np.asarray(inputs[f"g{i}"], dtype=f32).astype(bf16)
        base[f"be{i}"] = np.asarray(inputs[f"be{i}"], dtype=f32).astype(bf16)
    base["be2"] = (np.asarray(inputs["be2"], dtype=f32)
                   + np.asarray(inputs["be3"], dtype=f32)).astype(bf16)

    in_maps = []
    for c in range(B):
        m = dict(base)
        m["h"] = H[c].astype(bf16)
        m["at"] = np.ascontiguousarray(A[c].T).astype(bf16)
        in_maps.append(m)

    res = run_bass_kernel_spmd(nc, in_maps, list(range(B)))
    out = np.stack([res.results[c]["out"] for c in range(B)], axis=0)
    return out.astype(np.float32)


if __name__ == "__main__":
    nc = _get_program()
    print("program built ok")
    from concourse.timeline_sim import TimelineSim
    ns = TimelineSim(nc, trace=False).simulate()
    print(f"TimelineSim: {ns:.0f} ns")


# revision 67
# speedup vs baseline: 1.0282x; 1.0282x over previous
"""Trainium2 Bass kernel for nn_GRIC_31550829756424 (GCN-attention block).

Data-parallel over batch: 8 batches -> 8 NeuronCores, one full batch per core.
417581 ns (baseline) -> 133927 ns (TimelineSim, per core).

Key structure (v3):
- GCN reassociated: adjHnT = Hn^T @ adj_norm^T computed once (shared by
  Q/K/V), then Q/K/V are single-step K=128 matmuls.  A is host-transposed,
  bf16; deg^-1/2 folds into hn_t (partition side) + the adjHnT PSUM move
  (free side), so A^T itself is never rescaled.
- One activation table for the whole kernel (natural_log_exp_and_others,
  preloaded explicitly): every rsqrt is exp(-0.5*ln(x)); only Exp / Ln /
  Relu / Copy / Identity are ever used, so zero mid-kernel table reloads.
- Attention bias B added into the QK PSUM by an fp8e4 DoubleRow matmul
  (I/32 stationary x2 slots, bias*16 moving with a stride-0 slot dim) at
  0.5 cyc/row; B_bias ships as fp8 (halves its DMA traffic).
- Attention software-pipelined over heads: stage h runs scores+Exp(h) on
  PE/ACT while PV+LN(h-1) runs on PE/DVE/Pool; LN mean-subtract frees PSUM
  immediately, rstd is batched per half-head, MH transposed by DMA XBAR.
- LN1 gain/bias folded into W_O (host); LN2 gain/bias folded into W1/b1
  (host); residual H and the W_O/V biases accumulate on the PE via
  identity/ones-row matmuls.

Self-contained: hardcodes all shapes; imports only the in-container
concourse stack.
"""

import sys

sys.path.insert(0, "/opt/trn_rl_repo")

import numpy as np
import ml_dtypes
from contextlib import ExitStack

import concourse.bass as bass
import concourse.tile as tile
from concourse import bacc
from concourse import mybir
from concourse.bass_utils import run_bass_kernel_spmd
from concourse.masks import make_identity

F32 = mybir.dt.float32
BF16 = mybir.dt.bfloat16
F8 = mybir.dt.float8e4
AF = mybir.ActivationFunctionType
OP = mybir.AluOpType
PM = mybir.MatmulPerfMode

B = 8
N = 1024
D = 128
HEADS = 8
DV = 128
HD = HEADS * DV  # 1024
P = 128
NT = N // P  # 8 tiles of 128 rows
DK = 1.0 / float(np.sqrt(np.float32(D)))
EPS = 1e-5

_prog_cache = {}


def _bcast_load(nc, dst, src):
    """DMA-load 1D DRAM vector src [W] replicated across all P partitions of
    dst [P, W] (same dtype)."""
    rep = bass.AP(tensor=src.tensor, offset=src.offset, ap=[[0, P]] + list(src.ap))
    nc.gpsimd.dma_start(out=dst, in_=rep)


def _dup2(ap):
    """View a [P, W] AP as [P, 2, W] with a stride-0 middle dim (DoubleRow
    moving operand reading the same data in both slots)."""
    return bass.AP(
        tensor=ap.tensor, offset=ap.offset,
        ap=[list(ap.ap[0]), [0, 2]] + [list(a) for a in ap.ap[1:]],
    )


def _build_program():
    nc = bacc.Bacc(None)

    h_in = nc.declare_dram_parameter("h", [N, D], BF16, isOutput=False)
    at_in = nc.declare_dram_parameter("at", [N, N], BF16, isOutput=False)
    bt_in = nc.declare_dram_parameter("bt", [HEADS, N, N], F8, isOutput=False)
    wq_in = nc.declare_dram_parameter("wq", [D, HD], BF16, isOutput=False)
    wk_in = nc.declare_dram_parameter("wk", [D, HD], BF16, isOutput=False)
    wv_in = nc.declare_dram_parameter("wv", [D, HD], BF16, isOutput=False)
    bqr_in = nc.declare_dram_parameter("bqr", [P, NT], F32, isOutput=False)
    bkr_in = nc.declare_dram_parameter("bkr", [P, NT], F32, isOutput=False)
    bv_in = nc.declare_dram_parameter("bv", [HD], F32, isOutput=False)
    wo_in = nc.declare_dram_parameter("wo", [HD, D], BF16, isOutput=False)
    bo_in = nc.declare_dram_parameter("bo", [1, D], BF16, isOutput=False)
    w1_in = nc.declare_dram_parameter("w1", [D, D], BF16, isOutput=False)
    w2_in = nc.declare_dram_parameter("w2", [D, D], BF16, isOutput=False)
    b1_in = nc.declare_dram_parameter("b1", [D, 1], F32, isOutput=False)
    b2_in = nc.declare_dram_parameter("b2", [D, 1], F32, isOutput=False)
    g_in = {}
    be_in = {}
    for i in (0, 2, 3):
        g_in[i] = nc.declare_dram_parameter(f"g{i}", [D], BF16, isOutput=False)
        be_in[i] = nc.declare_dram_parameter(f"be{i}", [D], BF16, isOutput=False)
    out_dram = nc.declare_dram_parameter("out", [N, D], F32, isOutput=True)

    with tile.TileContext(nc) as tc, ExitStack() as ctx:
        consts = ctx.enter_context(tc.tile_pool(name="consts", bufs=1))
        persist = ctx.enter_context(tc.tile_pool(name="persist", bufs=1))
        small = ctx.enter_context(tc.tile_pool(name="small", bufs=12))
        stg = ctx.enter_context(tc.tile_pool(name="stg", bufs=3))
        # 2-bank 512-wide PSUM pool: open through B/C, reused in E via scope.
        ps512 = ctx.enter_context(
            tc.tile_pool(name="ps512", bufs=2, space=bass.MemorySpace.PSUM))

        # ---- constants -------------------------------------------------
        identb = consts.tile([P, P], BF16)
        make_identity(nc, identb)
        omib = consts.tile([P, P], BF16)  # 1 - I
        nc.gpsimd.memset(omib, 1.0)
        nc.gpsimd.affine_select(
            out=omib, in_=omib, compare_op=OP.not_equal, fill=0.0,
            base=0, pattern=[[-1, P]], channel_multiplier=1)
        # fp8 DoubleRow stationary: two slots of I/32 (moving is bias*16).
        id2 = consts.tile([P, 2, P], F8)
        nc.gpsimd.memset(id2, 0.0)
        for s in range(2):
            nc.gpsimd.affine_select(
                out=id2[:, s, :], in_=id2[:, s, :], compare_op=OP.not_equal,
                fill=1.0 / 32.0, base=0, pattern=[[-1, P]], channel_multiplier=1)
        nc.scalar.add_instruction(
            mybir.InstLoadActFuncSet(
                name=nc.get_next_instruction_name(), ins=[], outs=[],
                act_func_set_id=6))
        eps_t = consts.tile([P, 1], F32)
        nc.vector.memset(eps_t, EPS)
        onesb = consts.tile([1, P], BF16)
        nc.vector.memset(onesb, 1.0)
        onescol = consts.tile([P, 1], BF16)
        nc.vector.memset(onescol, 1.0)

        gb = {}
        beb = {}
        for i in (0, 2, 3):
            gb[i] = consts.tile([P, D], BF16, name=f"g{i}b", tag=f"g{i}b")
            beb[i] = consts.tile([P, D], BF16, name=f"be{i}b", tag=f"be{i}b")
        _bcast_load(nc, gb[0], g_in[0][:])
        _bcast_load(nc, beb[0], be_in[0][:])
        wq_sb = consts.tile([P, HD], BF16)
        nc.gpsimd.dma_start(out=wq_sb, in_=wq_in[:, :])
        wk_sb = consts.tile([P, HD], BF16)
        nc.gpsimd.dma_start(out=wk_sb, in_=wk_in[:, :])
        wv_sb = consts.tile([P, HD], BF16)
        nc.gpsimd.dma_start(out=wv_sb, in_=wv_in[:, :])
        bqr = consts.tile([P, NT], F32)
        nc.gpsimd.dma_start(out=bqr, in_=bqr_in[:, :])
        bkr = consts.tile([P, NT], F32)
        nc.gpsimd.dma_start(out=bkr, in_=bkr_in[:, :])
        bvb = consts.tile([P, HD], F32)
        _bcast_load(nc, bvb, bv_in[:])
        for i in (2, 3):
            _bcast_load(nc, gb[i], g_in[i][:])
            _bcast_load(nc, beb[i], be_in[i][:])
        # E-phase weights last: not needed until ~100us in
        wo_sb = consts.tile([P, HEADS, D], BF16)
        nc.gpsimd.dma_start(
            out=wo_sb, in_=wo_in.rearrange("(hh p) d -> p hh d", p=P))
        bo_sb = consts.tile([1, D], BF16)
        nc.gpsimd.dma_start(out=bo_sb, in_=bo_in[:, :])
        w1_sb = consts.tile([P, D], BF16)
        nc.gpsimd.dma_start(out=w1_sb, in_=w1_in[:, :])
        w2_sb = consts.tile([P, D], BF16)
        nc.gpsimd.dma_start(out=w2_sb, in_=w2_in[:, :])
        b1_sb = consts.tile([P, 1], F32)
        nc.gpsimd.dma_start(out=b1_sb, in_=b1_in[:, :])
        b2_sb = consts.tile([P, 1], F32)
        nc.gpsimd.dma_start(out=b2_sb, in_=b2_in[:, :])

        # ---- persistent tensors ---------------------------------------
        h_sb = persist.tile([P, NT, D], BF16, tag="h")
        hn_t = persist.tile([P, NT, D], BF16, tag="hnt")
        ats = persist.tile([P, NT, N], BF16, tag="ats")  # A^T -> adj_norm^T
        for j in range(NT):
            nc.sync.dma_start(
                out=ats[:, j, :],
                in_=at_in[:, :].rearrange("(t p) n -> p t n", p=P)[:, j, :])
        nc.sync.dma_start(out=h_sb, in_=h_in.rearrange("(t p) d -> p t d", p=P))
        disb = persist.tile([P, N], BF16, tag="disb")
        dis_tok = persist.tile([P, NT], F32, tag="distok")
        adjHnT = persist.tile([P, N], BF16, tag="adjhnt")
        qT = persist.tile([P, HEADS, N], BF16, tag="qT")
        kT = persist.tile([P, HEADS, N], BF16, tag="kT")
        vna = persist.tile([P, NT, HEADS, DV + 1], BF16, tag="v")
        nc.vector.memset(vna[:, :, :, DV:DV + 1], 1.0)
        mhcT = persist.tile([P, HEADS, N], BF16, tag="mhcT")

        # ---- phase B part 1: diagonal fix + rowsums ---------------------
        with tc.tile_pool(name="psR", bufs=1, space=bass.MemorySpace.PSUM) as psR, \
             tc.tile_pool(name="psTb", bufs=2, space=bass.MemorySpace.PSUM) as psTb:
            rs_ps = [psR.tile([1, 512], F32, name=f"rsps{c}", tag=f"rsps{c}")
                     for c in range(2)]
            for j in range(NT):
                db = ats[:, j, j * P:(j + 1) * P]
                nc.vector.tensor_mul(out=db, in0=db, in1=omib)
                nc.vector.tensor_add(out=db, in0=db, in1=identb)
                for c in range(2):
                    nc.tensor.matmul(
                        rs_ps[c], onescol, ats[:, j, c * 512:(c + 1) * 512],
                        start=(j == 0), stop=(j == NT - 1))
            # -- phase A: H LayerNorm (interleaved) --
            for i in range(NT):
                s6 = small.tile([P, 6], F32, tag="s6")
                mv = small.tile([P, 2], F32, tag="mv")
                nc.vector.bn_stats(out=s6, in_=h_sb[:, i, :])
                nc.vector.bn_aggr(out=mv, in_=s6)
                lnv = small.tile([P, 1], F32, tag="lnv")
                nc.scalar.activation(out=lnv, in_=mv[:, 1:2], func=AF.Ln, bias=eps_t)
                rstd = small.tile([P, 1], F32, tag="rstd")
                nc.scalar.activation(out=rstd, in_=lnv, func=AF.Exp, scale=-0.5)
                hw = stg.tile([P, D], BF16, name="hw", tag="hw")
                nc.vector.tensor_scalar(
                    out=hw, in0=h_sb[:, i, :], scalar1=mv[:, 0:1], scalar2=rstd,
                    op0=OP.subtract, op1=OP.mult)
                nc.vector.tensor_mul(out=hw, in0=hw, in1=gb[0])
                nc.vector.tensor_add(out=hn_t[:, i, :], in0=hw, in1=beb[0])

            rs_sb = small.tile([1, N], F32, tag="rssb", bufs=1)
            lnr = small.tile([1, N], F32, tag="lnr", bufs=1)
            disrow = small.tile([1, N], BF16, tag="disrow", bufs=1)
            for c in range(2):
                sl = slice(c * 512, (c + 1) * 512)
                nc.vector.tensor_scalar_max(
                    out=rs_sb[:, sl], in0=rs_ps[c], scalar1=1.0)
                nc.scalar.activation(out=lnr[:, sl], in_=rs_sb[:, sl],
                                     func=AF.Ln)
                nc.scalar.activation(out=disrow[:, sl], in_=lnr[:, sl],
                                     func=AF.Exp, scale=-0.5)
            # broadcast di over partitions: disb[q, n] = di_n
            for c in range(2):
                psd = ps512.tile([P, 512], F32, tag="ps512")
                nc.tensor.matmul(
                    psd, onesb, disrow[:, c * 512:(c + 1) * 512],
                    start=True, stop=True)
                nc.scalar.activation(
                    out=disb[:, c * 512:(c + 1) * 512], in_=psd, func=AF.Copy)
            # dis_tok[p, j] = di_{j*P+p} via PE transpose of disb chunks
            for j in range(NT):
                ptb = psTb.tile([P, P], BF16, tag="ptb")
                nc.tensor.transpose(ptb, disb[:, j * P:(j + 1) * P], identb)
                nc.scalar.activation(out=dis_tok[:, j:j + 1], in_=ptb[:, 0:1],
                                     func=AF.Copy)
            # fold di_m into hn_t (per-partition) instead of scaling ats
            for j in range(NT):
                nc.vector.tensor_scalar_mul(
                    out=hn_t[:, j, :], in0=hn_t[:, j, :],
                    scalar1=dis_tok[:, j:j + 1])

        # ---- phase C: shared GCN trunk ----------------------------------
        # adjHnT[d, n] = sum_m hn[m, d] * adjn^T[m, n]
        for c in range(2):
            psc = ps512.tile([P, 512], F32, tag="ps512")
            for j in range(NT):
                nc.tensor.matmul(
                    psc, hn_t[:, j, :], ats[:, j, c * 512:(c + 1) * 512],
                    start=(j == 0), stop=(j == NT - 1))
            nc.vector.tensor_mul(
                out=adjHnT[:, c * 512:(c + 1) * 512], in0=psc,
                in1=disb[:, c * 512:(c + 1) * 512])

        def emit_qk(hh, q_on_act=False):
            for c in range(2):
                psc = ps512.tile([P, 512], F32, tag="ps512")
                nc.tensor.matmul(
                    psc, wq_sb[:, hh * P:(hh + 1) * P],
                    adjHnT[:, c * 512:(c + 1) * 512], start=True, stop=True)
                if q_on_act:
                    nc.scalar.activation(
                        out=qT[:, hh, c * 512:(c + 1) * 512], in_=psc,
                        func=AF.Identity, bias=bqr[:, hh:hh + 1])
                else:
                    nc.vector.tensor_scalar_add(
                        out=qT[:, hh, c * 512:(c + 1) * 512], in0=psc,
                        scalar1=bqr[:, hh:hh + 1])
                psc = ps512.tile([P, 512], F32, tag="ps512")
                nc.tensor.matmul(
                    psc, wk_sb[:, hh * P:(hh + 1) * P],
                    adjHnT[:, c * 512:(c + 1) * 512], start=True, stop=True)
                nc.vector.tensor_scalar_add(
                    out=kT[:, hh, c * 512:(c + 1) * 512], in0=psc,
                    scalar1=bkr[:, hh:hh + 1])

        emit_qk(0, q_on_act=True)

        def emit_v(c, i0=0, i1=NT):
            for i in range(i0, i1):
                psc = ps512.tile([P, 512], F32, tag="ps512")
                nc.tensor.matmul(
                    psc, adjHnT[:, i * P:(i + 1) * P],
                    wv_sb[:, c * 512:(c + 1) * 512], start=True, stop=True)
                nc.vector.tensor_add(
                    out=vna[:, i, c * 4:(c + 1) * 4, 0:DV],
                    in0=psc.rearrange("p (a b) -> p a b", a=4),
                    in1=bvb[:, c * 512:(c + 1) * 512].rearrange(
                        "p (a b) -> p a b", a=4))

        emit_v(0)

        # ---- phase D: attention, software-pipelined over heads ----------
        with tc.tile_pool(name="psE", bufs=2, space=bass.MemorySpace.PSUM) as psE, \
             tc.tile_pool(name="psPM", bufs=2, space=bass.MemorySpace.PSUM) as psPM, \
             tc.tile_pool(name="etp", bufs=3) as etp, \
             tc.tile_pool(name="btp", bufs=2) as btp, \
             tc.tile_pool(name="mhp", bufs=3) as mhp:
            ets = {}
            mhs = {}
            mvss = {}
            vees = {}
            bt0 = btp.tile([P, NT, N], F8, tag="bt", name="bt0")
            nc.sync.dma_start(
                out=bt0, in_=bt_in[0].rearrange("(t p) n -> p t n", p=P))
            bts = {0: bt0}
            for stage in range(HEADS + 1):
                hh = stage
                if hh < HEADS:
                    et = etp.tile([P, NT, N], BF16, tag="et")
                    ets[hh] = et
                    btile = bts[hh]
                    for j in range(NT):
                        pse = psE.tile([P, N], F32, tag="pse")
                        for c in range(2):
                            nc.tensor.matmul(
                                pse[:, c * 512:(c + 1) * 512],
                                kT[:, hh, j * P:(j + 1) * P],
                                qT[:, hh, c * 512:(c + 1) * 512],
                                start=True, stop=False)
                            nc.tensor.matmul(
                                pse[:, c * 512:(c + 1) * 512],
                                id2, _dup2(btile[:, j, c * 512:(c + 1) * 512]),
                                start=False, stop=True, perf_mode=PM.DoubleRow)
                        nc.scalar.activation(out=et[:, j, :], in_=pse,
                                             func=AF.Exp)
                    if hh + 1 < HEADS:
                        btn = btp.tile([P, NT, N], F8, tag="bt",
                                       name=f"bt{hh + 1}")
                        nc.sync.dma_start(
                            out=btn,
                            in_=bt_in[hh + 1].rearrange("(t p) n -> p t n", p=P))
                        bts[hh + 1] = btn
                        if hh + 1 < HEADS:
                            emit_qk(hh + 1)
                if 1 <= stage <= 4:
                    emit_v(1, (stage - 1) * 2, stage * 2)
                if stage >= 1:
                    ph = stage - 1  # head whose PV/LN we process now
                    et = ets.pop(ph)
                    mh_sub = mhp.tile([P, NT, DV], BF16, tag="mh")
                    mhs[ph] = mh_sub
                    mvs = small.tile([P, NT, 2], F32, tag="mvs", bufs=2)
                    mvss[ph] = mvs
                    vee = small.tile([P, NT], F32, tag="vee", bufs=2)
                    vees[ph] = vee
                    for i in range(NT):
                        pm = psPM.tile([P, DV + 1], F32, tag="pm")
                        for j in range(NT):
                            nc.tensor.matmul(
                                pm, et[:, j, i * P:(i + 1) * P],
                                vna[:, j, ph, :],
                                start=(j == 0), stop=(j == NT - 1))
                        s6 = small.tile([P, 6], F32, tag="s6")
                        nc.vector.bn_stats(out=s6, in_=pm[:, 0:DV])
                        nc.vector.bn_aggr(out=mvs[:, i, :], in_=s6)
                        t = small.tile([P, 1], F32, tag="t")
                        nc.vector.tensor_scalar(
                            out=t, in0=pm[:, DV:DV + 1],
                            scalar1=pm[:, DV:DV + 1], scalar2=EPS,
                            op0=OP.mult, op1=OP.mult)
                        nc.gpsimd.tensor_add(
                            out=vee[:, i:i + 1], in0=t, in1=mvs[:, i, 1:2])
                        # mean-subtract now (frees pm); rstd scale later
                        nc.vector.tensor_scalar(
                            out=mh_sub[:, i, :], in0=pm[:, 0:DV],
                            scalar1=mvs[:, i, 0:1], scalar2=None,
                            op0=OP.subtract)
                    # rstd batched per half-head: first scales/transposes
                    # start before the second half's PV stats finish
                    lnv8 = small.tile([P, NT], F32, tag="lnv8", bufs=2)
                    rstd8 = small.tile([P, NT], F32, tag="rstd8", bufs=2)
                    eng = nc.vector if ph == HEADS - 1 else nc.gpsimd
                    for half in range(2):
                        sl = slice(half * 4, (half + 1) * 4)
                        nc.scalar.activation(out=lnv8[:, sl], in_=vee[:, sl],
                                             func=AF.Ln)
                        nc.scalar.activation(out=rstd8[:, sl], in_=lnv8[:, sl],
                                             func=AF.Exp, scale=-0.5)
                        for i in range(half * 4, (half + 1) * 4):
                            eng.tensor_scalar_mul(
                                out=mh_sub[:, i, :], in0=mh_sub[:, i, :],
                                scalar1=rstd8[:, i:i + 1])
                        for qt in range(half * 2, (half + 1) * 2):
                            nc.sync.dma_start(
                                out=mhcT[:, ph, qt * 256:(qt + 1) * 256]
                                .rearrange("p (t f) -> p t f", t=2),
                                in_=mh_sub[:, qt * 2:(qt + 1) * 2, :],
                                transpose=True)

        # ---- phase E: output projection + MLP ---------------------------
        o_ln = persist.tile([P, NT, D], BF16, tag="oln")
        obe = persist.tile([P, NT, D], BF16, tag="obe")
        mvE = small.tile([P, NT, 2], F32, tag="mvE", bufs=1)
        rstdE = small.tile([P, NT], F32, tag="rstdE", bufs=1)
        with tc.tile_pool(name="psO", bufs=4, space=bass.MemorySpace.PSUM) as psO, \
             tc.tile_pool(name="psTe", bufs=2, space=bass.MemorySpace.PSUM) as psTe:
            oT = persist.tile([P, NT, P], BF16, tag="oT")
            for i in range(NT):
                pso = psO.tile([P, D], F32, tag="pso")
                for hh in range(HEADS):
                    nc.tensor.matmul(
                        pso, mhcT[:, hh, i * P:(i + 1) * P], wo_sb[:, hh, :],
                        start=(hh == 0), stop=False)
                nc.tensor.matmul(pso, onesb, bo_sb, start=False, stop=False)
                nc.tensor.matmul(pso, identb, h_sb[:, i, :],
                                 start=False, stop=True)
                s6 = small.tile([P, 6], F32, tag="s6")
                nc.vector.bn_stats(out=s6, in_=pso)
                nc.vector.bn_aggr(out=mvE[:, i, :], in_=s6)
                lnv = small.tile([P, 1], F32, tag="lnv")
                nc.scalar.activation(out=lnv, in_=mvE[:, i, 1:2], func=AF.Ln,
                                     bias=eps_t)
                nc.scalar.activation(out=rstdE[:, i:i + 1], in_=lnv,
                                     func=AF.Exp, scale=-0.5)
                nc.vector.tensor_scalar(
                    out=o_ln[:, i, :], in0=pso,
                    scalar1=mvE[:, i, 0:1], scalar2=rstdE[:, i:i + 1],
                    op0=OP.subtract, op1=OP.mult)
                ptb = psTe.tile([P, P], BF16, tag="pte")
                nc.tensor.transpose(ptb, o_ln[:, i, :], identb)
                nc.scalar.activation(out=oT[:, i, :], in_=ptb, func=AF.Copy)
                nc.gpsimd.tensor_mul(out=obe[:, i, :], in0=o_ln[:, i, :],
                                     in1=gb[2])
                nc.gpsimd.tensor_add(out=obe[:, i, :], in0=obe[:, i, :],
                                     in1=beb[2])
            r1T = persist.tile([P, N], BF16, tag="r1T")
            for c in range(2):
                psc = ps512.tile([P, 512], F32, tag="ps512")
                nc.tensor.matmul(
                    psc, w1_sb,
                    oT[:, 4 * c:4 * (c + 1), :].rearrange("p t f -> p (t f)"),
                    start=True, stop=True)
                nc.scalar.activation(
                    out=r1T[:, c * 512:(c + 1) * 512], in_=psc, func=AF.Relu,
                    bias=b1_sb)
            r2T = persist.tile([P, N], BF16, tag="r2T")
            for c in range(2):
                psc = ps512.tile([P, 512], F32, tag="ps512")
                nc.tensor.matmul(
                    psc, w2_sb, r1T[:, c * 512:(c + 1) * 512],
                    start=True, stop=True)
                nc.scalar.activation(
                    out=r2T[:, c * 512:(c + 1) * 512], in_=psc, func=AF.Relu,
                    bias=b2_sb)

            r2tok = persist.tile([P, NT, D], BF16, tag="r2tok")
            mvR = small.tile([P, NT, 2], F32, tag="mvR", bufs=1)
            rstdR = small.tile([P, NT], F32, tag="rstdR", bufs=1)
            out_sb = persist.tile([P, NT, D], F32, tag="osb")
            for i in range(NT):
                ptb = psTe.tile([P, P], BF16, tag="pte")
                nc.tensor.transpose(ptb, r2T[:, i * P:(i + 1) * P], identb)
                nc.scalar.activation(out=r2tok[:, i, :], in_=ptb, func=AF.Copy)
                s6 = small.tile([P, 6], F32, tag="s6")
                nc.vector.bn_stats(out=s6, in_=r2tok[:, i, :])
                nc.vector.bn_aggr(out=mvR[:, i, :], in_=s6)
                lnv = small.tile([P, 1], F32, tag="lnv")
                nc.scalar.activation(out=lnv, in_=mvR[:, i, 1:2], func=AF.Ln,
                                     bias=eps_t)
                nc.scalar.activation(out=rstdR[:, i:i + 1], in_=lnv,
                                     func=AF.Exp, scale=-0.5)
            for i in range(NT):
                ro = stg.tile([P, D], BF16, name="ro", tag="ro")
                nc.vector.tensor_scalar(
                    out=ro, in0=r2tok[:, i, :], scalar1=mvR[:, i, 0:1],
                    scalar2=rstdR[:, i:i + 1], op0=OP.subtract, op1=OP.mult)
                eng = nc.gpsimd if i % 2 == 0 else nc.vector
                eng.tensor_mul(out=ro, in0=ro, in1=gb[3])
                eng.tensor_add(out=out_sb[:, i, :], in0=obe[:, i, :],
                               in1=ro)
                if i in (3, 5, 6, 7):
                    lo = {3: 0, 5: 4, 6: 6, 7: 7}[i]
                    nc.sync.dma_start(
                        out=out_dram.rearrange(
                            "(t p) d -> p t d", p=P)[:, lo:i + 1, :],
                        in_=out_sb[:, lo:i + 1, :])

    nc.compile()
    return nc


def _get_program():
    if "nc" not in _prog_cache:
        _prog_cache["nc"] = _build_program()
    return _prog_cache["nc"]


def kernel(**inputs):
    nc = _get_program()
    f32 = np.float32
    bf16 = ml_dtypes.bfloat16
    f8 = ml_dtypes.float8_e4m3fn

    H = np.asarray(inputs["H"], dtype=f32)
    A = np.asarray(inputs["A"], dtype=f32)
    g1 = np.asarray(inputs["g1"], dtype=f32)
    be1 = np.asarray(inputs["be1"], dtype=f32)
    WO = np.asarray(inputs["W_O"], dtype=f32)
    # fold LN1 gain/bias into the output projection
    WO_fold = WO * np.tile(g1, HEADS)[:, None]
    bO = np.tile(be1, HEADS) @ WO

    BT = np.asarray(inputs["B_bias"], dtype=f32).transpose(0, 2, 1)
    base = {
        "bt": np.ascontiguousarray(BT * 16.0).astype(f8),
        "wq": (np.asarray(inputs["W_Q"], dtype=f32) * DK).astype(bf16),
        "wk": np.asarray(inputs["W_K"], dtype=f32).astype(bf16),
        "wv": np.asarray(inputs["W_V"], dtype=f32).astype(bf16),
        "bqr": np.ascontiguousarray(
            (np.asarray(inputs["b_Q"], dtype=f32) * DK).reshape(NT, P).T),
        "bkr": np.ascontiguousarray(
            np.asarray(inputs["b_K"], dtype=f32).reshape(NT, P).T),
        "bv": np.asarray(inputs["b_V"], dtype=f32),
        "wo": WO_fold.astype(bf16),
        "bo": bO.reshape(1, D).astype(bf16),
        "w1": (np.asarray(inputs["g2"], dtype=f32)[:, None]
               * np.asarray(inputs["W1"], dtype=f32)).astype(bf16),
        "w2": np.asarray(inputs["W2"], dtype=f32).astype(bf16),
        "b1": (np.asarray(inputs["b1"], dtype=f32)
               + np.asarray(inputs["be2"], dtype=f32)
               @ np.asarray(inputs["W1"], dtype=f32)).reshape(D, 1),
        "b2": np.asarray(inputs["b2"], dtype=f32).reshape(D, 1),
    }
    for i in (0, 2, 3):
        base[f"g{i}"] = np.asarray(inputs[f"g{i}"], dtype=f32).astype(bf16)
        base[f"be{i}"] = np.asarray(inputs[f"be{i}"], dtype=f32).astype(bf16)
    base["be2"] = (np.asarray(inputs["be2"], dtype=f32)
                   + np.asarray(inputs["be3"], dtype=f32)).astype(bf16)

    in_maps = []
    for c in range(B):
        m = dict(base)
        m["h"] = H[c].astype(bf16)
        m["at"] = np.ascontiguousarray(A[c].T).astype(bf16)
        in_maps.append(m)

    res = run_bass_kernel_spmd(nc, in_maps, list(range(B)))
    out = np.stack([res.results[c]["out"] for c in range(B)], axis=0)
    return out.astype(np.float32)


if __name__ == "__main__":
    nc = _get_program()
    print("program built ok")
    from concourse.timeline_sim import TimelineSim
    ns = TimelineSim(nc, trace=False).simulate()
    print(f"TimelineSim: {ns:.0f} ns")
